# Initial kernel scaffold
#
"""Causal attention (B=8, T=2048, D=1024, fp32) on 8 trn2 NeuronCores.

Sharding: data-parallel over batch — core b computes batch element b.
Per-core kernel (flash-style, causal block-skipped):
  S[q,k] = Q @ K^T          (TensorE, fp32r, d-major operands via PE transposes)
  P      = exp((S + mask)/sqrt(D))   (ScalarE, row-sums via accum_out)
  O      = (P @ V) / rowsum(P)       (TensorE + DVE normalize)
"""

import sys

if "/opt/trn_rl_repo" not in sys.path:
    sys.path.insert(0, "/opt/trn_rl_repo")

import numpy as np

B, T, D = 8, 2048, 1024
NQ = T // 128   # 16 query blocks of 128
NKC = T // 128  # 16 key chunks of 128
ND = D // 128   # 8 d chunks of 128
KTW = 512       # key tile width for S
NEG = -1e10
SOFTMAX_SCALE = 1.0 / float(np.sqrt(D))

_CACHE = {}


def _patch_drain():
    """This container's walrus accepts only one sync-wait per Drain
    instruction; Tile's kernel-tail drain accumulates one wait per ticked
    proc.  Split the extras onto follow-up drains (SP runs them in order,
    so every sem is still waited on before the end-of-kernel barrier)."""
    import concourse.mybir as mybir
    import concourse.tile as tile
    from concourse.vector_clock import ScopedClock

    if getattr(tile.TileContext, "_drain_split_patched", False):
        return

    def _drain_and_barrier_split(self, tick_clock, wait_clock):
        drain_inst = self.nc.sync.drain()
        wait_clock.add_sem_waits(
            drain_inst.ins, ScopedClock({None: tick_clock.global_clock})
        )
        si = drain_inst.ins.sync_info
        if si is not None and len(si.on_wait) > 1:
            waits = list(si.on_wait)
            drain_inst.ins.sync_info = mybir.SyncInfo(
                on_wait=[waits[0]], on_update=list(si.on_update)
            )
            for w in waits[1:]:
                extra = self.nc.sync.drain()
                extra.ins.sync_info = mybir.SyncInfo(on_wait=[w], on_update=[])

        self.nc.all_engine_barrier()
        assert self.sems is not None
        popped = self.nc._tile_sem_poison_stack.pop()
        assert popped is self._sem_poison
        self.nc.clear_and_free_semaphores(list(self.sems.allocated().values()))
        self.nc.all_engine_barrier()

    tile.TileContext._drain_and_barrier = _drain_and_barrier_split
    tile.TileContext._drain_split_patched = True


def _build():
    _patch_drain()
    import concourse.bass as bass
    import concourse.mybir as mybir
    import concourse.tile as tile
    from concourse import masks

    f32 = mybir.dt.float32
    f32r = mybir.dt.float32r
    EXP = mybir.ActivationFunctionType.Exp
    X = mybir.AxisListType.X

    nc = bass.Bass()
    q_d = nc.dram_tensor("query", [T, D], f32, kind="ExternalInput")
    k_d = nc.dram_tensor("key", [T, D], f32, kind="ExternalInput")
    v_d = nc.dram_tensor("value", [T, D], f32, kind="ExternalInput")
    o_d = nc.dram_tensor("out", [T, D], f32, kind="ExternalOutput")

    with tile.TileContext(nc) as tc:
        with (
            tc.tile_pool(name="const", bufs=1) as constp,
            tc.tile_pool(name="big", bufs=1) as bigp,
            tc.tile_pool(name="kstage", bufs=3) as kstagep,
            tc.tile_pool(name="qstage", bufs=2) as qstagep,
            tc.tile_pool(name="qt", bufs=2) as qtp,
            tc.tile_pool(name="p", bufs=3) as pp,
            tc.tile_pool(name="pt", bufs=3) as ptp,
            tc.tile_pool(name="osb", bufs=2) as osbp,
            tc.tile_pool(name="small", bufs=2) as smallp,
            tc.tile_pool(name="psum_s", bufs=2, space="PSUM") as psum_s,
            tc.tile_pool(name="psum_tr", bufs=2, space="PSUM") as psum_tr,
            tc.tile_pool(name="psum_o", bufs=2, space="PSUM") as psum_o,
        ):
            ident = constp.tile([128, 128], f32)
            masks.make_identity(nc, ident[:])
            ident_r = ident[:].bitcast(f32r)

            # Causal additive mask tiles for the diagonal k-tile.
            # cmask[p, ri, c] = 0 if c <= p + ri*128 else NEG
            cmask = constp.tile([128, 4, KTW], f32)
            nc.gpsimd.memset(cmask[:], 0.0)
            for ri in range(4):
                nc.gpsimd.affine_select(
                    out=cmask[:, ri, :],
                    in_=cmask[:, ri, :],
                    compare_op=mybir.AluOpType.is_ge,
                    fill=NEG,
                    base=ri * 128,
                    channel_multiplier=1,
                    pattern=[[-1, KTW]],
                )

            # V resident: v_all[p, kc, d]
            v_all = bigp.tile([128, NKC, D], f32)
            for kc in range(NKC):
                nc.sync.dma_start(v_all[:, kc, :], v_d[kc * 128:(kc + 1) * 128, :])

            # K^T resident: kt_all[p=d0, dc, t]
            kt_all = bigp.tile([128, ND, T], f32)
            for kc in range(NKC):
                kst = kstagep.tile([128, D], f32)
                nc.sync.dma_start(kst[:], k_d[kc * 128:(kc + 1) * 128, :])
                for g in range(2):
                    trp = psum_tr.tile([128, 512], f32)
                    for j in range(4):
                        dc = g * 4 + j
                        nc.tensor.transpose(
                            trp[:, j * 128:(j + 1) * 128],
                            kst[:, dc * 128:(dc + 1) * 128].bitcast(f32r),
                            ident_r,
                        )
                    nc.vector.tensor_copy(
                        kt_all[:, g * 4:(g + 1) * 4, kc * 128:(kc + 1) * 128],
                        trp[:].rearrange("p (a b) -> p a b", b=128),
                    )

            for qb in range(NQ):
                n_kc = qb + 1
                n_kt = (n_kc + 3) // 4

                qst = qstagep.tile([128, D], f32)
                nc.sync.dma_start(qst[:], q_d[qb * 128:(qb + 1) * 128, :])
                qt = qtp.tile([128, ND, 128], f32)
                for g in range(2):
                    trp = psum_tr.tile([128, 512], f32)
                    for j in range(4):
                        dc = g * 4 + j
                        nc.tensor.transpose(
                            trp[:, j * 128:(j + 1) * 128],
                            qst[:, dc * 128:(dc + 1) * 128].bitcast(f32r),
                            ident_r,
                        )
                    nc.vector.tensor_copy(
                        qt[:, g * 4:(g + 1) * 4, :],
                        trp[:].rearrange("p (a b) -> p a b", b=128),
                    )

                asum = smallp.tile([128, 16], f32, tag="asum")
                o_ps = psum_o.tile([128, D], f32)

                for kt in range(n_kt):
                    s_ps = psum_s.tile([128, KTW], f32)
                    for dc in range(ND):
                        nc.tensor.matmul(
                            s_ps[:],
                            qt[:, dc, :].bitcast(f32r),
                            kt_all[:, dc, kt * KTW:(kt + 1) * KTW].bitcast(f32r),
                            start=(dc == 0),
                            stop=(dc == ND - 1),
                        )
                    if kt == n_kt - 1:
                        ri = qb - (n_kt - 1) * 4
                        nc.vector.tensor_add(s_ps[:], s_ps[:], cmask[:, ri, :])

                    p_sb = pp.tile([128, KTW], f32)
                    nc.scalar.activation(
                        p_sb[:], s_ps[:], EXP,
                        bias=0.0, scale=SOFTMAX_SCALE,
                        accum_out=asum[:, kt:kt + 1],
                    )

                    n_j = min(4, n_kc - kt * 4)
                    pt_ps = psum_tr.tile([128, 512], f32)
                    pt_sb = ptp.tile([128, 512], f32)
                    for j in range(n_j):
                        nc.tensor.transpose(
                            pt_ps[:, j * 128:(j + 1) * 128],
                            p_sb[:, j * 128:(j + 1) * 128].bitcast(f32r),
                            ident_r,
                        )
                    nc.vector.tensor_copy(
                        pt_sb[:, :n_j * 128], pt_ps[:, :n_j * 128]
                    )
                    for j in range(n_j):
                        kc = kt * 4 + j
                        for h in range(2):
                            nc.tensor.matmul(
                                o_ps[:, h * 512:(h + 1) * 512],
                                pt_sb[:, j * 128:(j + 1) * 128].bitcast(f32r),
                                v_all[:, kc, h * 512:(h + 1) * 512].bitcast(f32r),
                                start=(kc == 0),
                                stop=(kc == n_kc - 1),
                            )

                rsum = smallp.tile([128, 1], f32, tag="rsum")
                nc.vector.reduce_sum(rsum[:], asum[:, :n_kt], axis=X)
                rinv = smallp.tile([128, 1], f32, tag="rinv")
                nc.vector.reciprocal(rinv[:], rsum[:])
                o_sb = osbp.tile([128, D], f32)
                nc.vector.tensor_scalar_mul(o_sb[:], o_ps[:], rinv[:])
                nc.sync.dma_start(o_d[qb * 128:(qb + 1) * 128, :], o_sb[:])

    return nc


def _np_reference(query, key, value, mask):
    """Host fallback for the general (non-all-ones) padding-mask case."""
    out = np.empty_like(query)
    tri = np.triu(np.ones((T, T), dtype=np.float32), 1) * 1e10
    for b in range(B):
        s = query[b] @ key[b].T
        s = s - tri
        s = s - (1.0 - mask[b])[None, :] * 1e10
        s = s * SOFTMAX_SCALE
        s = s - s.max(axis=-1, keepdims=True)
        p = np.exp(s)
        p = p / p.sum(axis=-1, keepdims=True)
        out[b] = p @ value[b]
    return out


def kernel(query, key, value, mask):
    query = np.ascontiguousarray(np.asarray(query, dtype=np.float32))
    key = np.ascontiguousarray(np.asarray(key, dtype=np.float32))
    value = np.ascontiguousarray(np.asarray(value, dtype=np.float32))
    mask = np.asarray(mask, dtype=np.float32)

    if not np.all(mask == 1.0):
        return _np_reference(query, key, value, mask)

    from concourse.bass_utils import run_bass_kernel_spmd

    if "nc" not in _CACHE:
        _CACHE["nc"] = _build()
    nc = _CACHE["nc"]

    in_maps = [
        {"query": query[b], "key": key[b], "value": value[b]}
        for b in range(B)
    ]
    res = run_bass_kernel_spmd(nc, in_maps, core_ids=list(range(B)))
    out = np.stack([res.results[b]["out"] for b in range(B)], axis=0)
    return out.astype(np.float32)


# revision 11
# speedup vs baseline: 1.0228x; 1.0228x over previous
"""Causal attention (B=8, T=2048, D=1024, fp32) on 8 trn2 NeuronCores.

Sharding: data-parallel over batch — core b computes batch element b.
Per-core kernel (flash-style, causal block-skipped):
  S[q,k] = Q @ K^T          (TensorE, fp32r, d-major operands via PE transposes)
  P      = exp((S + mask)/sqrt(D))   (ScalarE, row-sums via accum_out)
  O      = (P @ V) / rowsum(P)       (TensorE + DVE normalize)
"""

import sys

if "/opt/trn_rl_repo" not in sys.path:
    sys.path.insert(0, "/opt/trn_rl_repo")

import numpy as np

B, T, D = 8, 2048, 1024
NQ = T // 128   # 16 query blocks of 128
NKC = T // 128  # 16 key chunks of 128
ND = D // 128   # 8 d chunks of 128
KTW = 512       # key tile width for S
NEG = -1e10
SOFTMAX_SCALE = 1.0 / float(np.sqrt(D))

_CACHE = {}


def _split_waits(nc):
    """This container's walrus accepts only ONE sync-wait per instruction
    (setupSyncWait: 'Too many sync wait commands').  Tile freely attaches
    several waits to one instruction.  Hoist the extras onto same-engine
    NoOps inserted immediately before the instruction — each engine
    executes its stream in order, so the wait semantics are unchanged."""
    import concourse.mybir as mybir

    n_split = 0
    for f in nc.m.functions:
        for bb in f.blocks:
            out = []
            for inst in bb.instructions:
                si = inst.sync_info
                if si is not None and len(si.on_wait) > 1:
                    waits = list(si.on_wait)
                    for w in waits[:-1]:
                        nop = mybir.InstNoOp(
                            name=f"{inst.name}-w{n_split}",
                            engine=inst.engine,
                            sync_info=mybir.SyncInfo(on_wait=[w], on_update=[]),
                            bass_nofuse=True,
                        )
                        out.append(nop)
                        n_split += 1
                    inst.sync_info = mybir.SyncInfo(
                        on_wait=[waits[-1]], on_update=list(si.on_update)
                    )
                out.append(inst)
            bb.instructions[:] = out
    return n_split


def _build():
    import concourse.bass as bass
    import concourse.mybir as mybir
    import concourse.tile as tile
    from concourse import masks

    f32 = mybir.dt.float32
    f32r = mybir.dt.float32r
    EXP = mybir.ActivationFunctionType.Exp
    X = mybir.AxisListType.X

    nc = bass.Bass()
    q_d = nc.dram_tensor("query", [T, D], f32r, kind="ExternalInput")
    k_d = nc.dram_tensor("key", [T, D], f32r, kind="ExternalInput")
    v_d = nc.dram_tensor("value", [T, D], f32r, kind="ExternalInput")
    o_d = nc.dram_tensor("out", [T, D], f32, kind="ExternalOutput")

    with tile.TileContext(nc) as tc:
        with (
            tc.tile_pool(name="const", bufs=1) as constp,
            tc.tile_pool(name="big", bufs=1) as bigp,
            tc.tile_pool(name="kstage", bufs=3) as kstagep,
            tc.tile_pool(name="qstage", bufs=4) as qstagep,
            tc.tile_pool(name="qt", bufs=2) as qtp,
            tc.tile_pool(name="p", bufs=3) as pp,
            tc.tile_pool(name="pt", bufs=3) as ptp,
            tc.tile_pool(name="osb", bufs=2) as osbp,
            tc.tile_pool(name="small", bufs=2) as smallp,
            tc.tile_pool(name="psum_s", bufs=2, space="PSUM") as psum_s,
            tc.tile_pool(name="psum_tr", bufs=4, space="PSUM") as psum_tr,
            tc.tile_pool(name="psum_o", bufs=1, space="PSUM") as psum_o,
        ):
            ident_f = constp.tile([128, 128], f32)
            masks.make_identity(nc, ident_f[:])
            ident = constp.tile([128, 128], f32r)
            nc.vector.tensor_copy(ident[:], ident_f[:])
            ident_r = ident[:]

            # Causal additive mask tiles for the diagonal k-tile.
            # cmask[p, ri, c] = 0 if c <= p + ri*128 else NEG
            cmask = constp.tile([128, 4, KTW], f32)
            nc.gpsimd.memset(cmask[:], 0.0)
            for ri in range(4):
                nc.gpsimd.affine_select(
                    out=cmask[:, ri, :],
                    in_=cmask[:, ri, :],
                    compare_op=mybir.AluOpType.is_ge,
                    fill=NEG,
                    base=ri * 128,
                    channel_multiplier=1,
                    pattern=[[-1, KTW]],
                )

            v_all = bigp.tile([128, NKC, D], f32r)
            kt_all = bigp.tile([128, ND, T], f32r)

            # HAM heater: PE is otherwise idle during the first DMAs; a burst
            # of dummy matmuls flips the clock gate to 8/8 before real work.
            heat_ps = psum_s.tile([128, KTW], f32, tag="s_ps")
            for _ in range(40):
                nc.tensor.matmul(heat_ps[:, :128], ident[:], ident[:],
                                 start=True, stop=True)

            q_tiles = {}

            def issue_q(qb):
                if qb < NQ and qb not in q_tiles:
                    qst = qstagep.tile([128, D], f32r, tag="qst")
                    nc.sync.dma_start(qst[:], q_d[qb * 128:(qb + 1) * 128, :])
                    q_tiles[qb] = qst

            # ---- per-q-block stage emitters ------------------------------
            state = {}

            def emit_qt(qb):
                qst = q_tiles.pop(qb)
                qt = qtp.tile([128, ND, 128], f32r)
                for g in range(2):
                    trp = psum_tr.tile([128, 512], f32r, tag="tr")
                    for j in range(4):
                        dc = g * 4 + j
                        nc.tensor.transpose(
                            trp[:, j * 128:(j + 1) * 128],
                            qst[:, dc * 128:(dc + 1) * 128],
                            ident_r,
                        )
                    nc.vector.tensor_copy(
                        qt[:, g * 4:(g + 1) * 4, :],
                        trp[:].rearrange("p (a b) -> p a b", b=128),
                    )
                st = state[qb] = {}
                st["qt"] = qt
                asum_t = smallp.tile([128, 16], f32, tag="asum")
                st["asum"] = asum_t
                o_ps_t = psum_o.tile([128, D], f32, tag="ops")
                st["o_ps"] = o_ps_t
                st["p"] = {}

            def emit_qkt(qb, kt):
                st = state[qb]
                n_kc = qb + 1
                n_kt = (n_kc + 3) // 4
                qt = st["qt"]
                s_ps = psum_s.tile([128, KTW], f32, tag="s_ps")
                for dc in range(ND):
                    nc.tensor.matmul(
                        s_ps[:],
                        qt[:, dc, :],
                        kt_all[:, dc, kt * KTW:(kt + 1) * KTW],
                        start=(dc == 0),
                        stop=(dc == ND - 1),
                    )
                if kt == n_kt - 1:
                    ri = qb - (n_kt - 1) * 4
                    nc.vector.tensor_add(s_ps[:], s_ps[:], cmask[:, ri, :])
                p_sb = pp.tile([128, KTW], f32r)
                nc.scalar.activation(
                    p_sb[:], s_ps[:], EXP,
                    bias=0.0, scale=SOFTMAX_SCALE,
                    accum_out=st["asum"][:, kt:kt + 1],
                )
                st["p"][kt] = p_sb

            def emit_ptpv(qb, kt):
                st = state[qb]
                n_kc = qb + 1
                p_sb = st["p"].pop(kt)
                o_ps = st["o_ps"]
                n_j = min(4, n_kc - kt * 4)
                pt_ps = psum_tr.tile([128, 512], f32r, tag="tr")
                pt_sb = ptp.tile([128, 512], f32r)
                for j in range(n_j):
                    nc.tensor.transpose(
                        pt_ps[:, j * 128:(j + 1) * 128],
                        p_sb[:, j * 128:(j + 1) * 128],
                        ident_r,
                    )
                nc.vector.tensor_copy(pt_sb[:, :n_j * 128], pt_ps[:, :n_j * 128])
                for j in range(n_j):
                    kc = kt * 4 + j
                    for h in range(2):
                        nc.tensor.matmul(
                            o_ps[:, h * 512:(h + 1) * 512],
                            pt_sb[:, j * 128:(j + 1) * 128],
                            v_all[:, kc, h * 512:(h + 1) * 512],
                            start=(kc == 0),
                            stop=(kc == n_kc - 1),
                        )

            def emit_fin(qb):
                st = state.pop(qb)
                n_kt = (qb + 4) // 4
                rsum = smallp.tile([128, 1], f32, tag="rsum")
                nc.vector.reduce_sum(rsum[:], st["asum"][:, :n_kt], axis=X)
                rinv = smallp.tile([128, 1], f32, tag="rinv")
                nc.vector.reciprocal(rinv[:], rsum[:])
                o_sb = osbp.tile([128, D], f32)
                nc.scalar.mul(o_sb[:], st["o_ps"][:], rinv[:])
                nc.sync.dma_start(o_d[qb * 128:(qb + 1) * 128, :], o_sb[:])

            def emit_qblock(qb):
                n_kt = (qb + 4) // 4
                for kt in range(n_kt):
                    emit_qkt(qb, kt)
                    if kt >= 1:
                        emit_ptpv(qb, kt - 1)
                if qb + 1 < NQ:
                    emit_qt(qb + 1)  # next block's Q^T: casts overlap PV tail
                emit_ptpv(qb, n_kt - 1)
                emit_fin(qb)

            # ---- merged pipeline: K load/transpose + staggered q-blocks --
            issue_q(0)
            issue_q(1)
            for kc in range(NKC):
                kst = kstagep.tile([128, D], f32r)
                nc.sync.dma_start(kst[:], k_d[kc * 128:(kc + 1) * 128, :])
                issue_q(kc + 2)
                nc.sync.dma_start(v_all[:, kc, :], v_d[kc * 128:(kc + 1) * 128, :])
                for g in range(2):
                    trp = psum_tr.tile([128, 512], f32r, tag="tr")
                    for j in range(4):
                        dc = g * 4 + j
                        nc.tensor.transpose(
                            trp[:, j * 128:(j + 1) * 128],
                            kst[:, dc * 128:(dc + 1) * 128],
                            ident_r,
                        )
                    nc.vector.tensor_copy(
                        kt_all[:, g * 4:(g + 1) * 4, kc * 128:(kc + 1) * 128],
                        trp[:].rearrange("p (a b) -> p a b", b=128),
                    )
                if kc == 0:
                    emit_qt(0)
                if kc >= 1:
                    emit_qblock(kc - 1)
            emit_qblock(NQ - 1)

    _split_waits(nc)
    return nc


def _np_reference(query, key, value, mask):
    """Host fallback for the general (non-all-ones) padding-mask case."""
    out = np.empty_like(query)
    tri = np.triu(np.ones((T, T), dtype=np.float32), 1) * 1e10
    for b in range(B):
        s = query[b] @ key[b].T
        s = s - tri
        s = s - (1.0 - mask[b])[None, :] * 1e10
        s = s * SOFTMAX_SCALE
        s = s - s.max(axis=-1, keepdims=True)
        p = np.exp(s)
        p = p / p.sum(axis=-1, keepdims=True)
        out[b] = p @ value[b]
    return out


def kernel(query, key, value, mask):
    query = np.ascontiguousarray(np.asarray(query, dtype=np.float32))
    key = np.ascontiguousarray(np.asarray(key, dtype=np.float32))
    value = np.ascontiguousarray(np.asarray(value, dtype=np.float32))
    mask = np.asarray(mask, dtype=np.float32)

    if not np.all(mask == 1.0):
        return _np_reference(query, key, value, mask)

    from concourse.bass_utils import run_bass_kernel_spmd

    if "nc" not in _CACHE:
        _CACHE["nc"] = _build()
    nc = _CACHE["nc"]

    in_maps = [
        {"query": query[b], "key": key[b], "value": value[b]}
        for b in range(B)
    ]
    res = run_bass_kernel_spmd(nc, in_maps, core_ids=list(range(B)))
    out = np.stack([res.results[b]["out"] for b in range(B)], axis=0)
    return out.astype(np.float32)


# revision 12
# speedup vs baseline: 1.0275x; 1.0046x over previous
"""Causal attention (B=8, T=2048, D=1024, fp32) on 8 trn2 NeuronCores.

Sharding: data-parallel over batch — core b computes batch element b.
Per-core kernel (flash-style, causal block-skipped):
  S[q,k] = Q @ K^T          (TensorE, fp32r, d-major operands via PE transposes)
  P      = exp((S + mask)/sqrt(D))   (ScalarE, row-sums via accum_out)
  O      = (P @ V) / rowsum(P)       (TensorE + DVE normalize)
"""

import sys

if "/opt/trn_rl_repo" not in sys.path:
    sys.path.insert(0, "/opt/trn_rl_repo")

import numpy as np

B, T, D = 8, 2048, 1024
NQ = T // 128   # 16 query blocks of 128
NKC = T // 128  # 16 key chunks of 128
ND = D // 128   # 8 d chunks of 128
KTW = 512       # key tile width for S
NEG = -1e10
SOFTMAX_SCALE = 1.0 / float(np.sqrt(D))

_CACHE = {}


def _split_waits(nc):
    """This container's walrus accepts only ONE sync-wait per instruction
    (setupSyncWait: 'Too many sync wait commands').  Tile freely attaches
    several waits to one instruction.  Hoist the extras onto same-engine
    NoOps inserted immediately before the instruction — each engine
    executes its stream in order, so the wait semantics are unchanged."""
    import concourse.mybir as mybir

    n_split = 0
    for f in nc.m.functions:
        for bb in f.blocks:
            out = []
            for inst in bb.instructions:
                si = inst.sync_info
                if si is not None and len(si.on_wait) > 1:
                    waits = list(si.on_wait)
                    for w in waits[:-1]:
                        nop = mybir.InstNoOp(
                            name=f"{inst.name}-w{n_split}",
                            engine=inst.engine,
                            sync_info=mybir.SyncInfo(on_wait=[w], on_update=[]),
                            bass_nofuse=True,
                        )
                        out.append(nop)
                        n_split += 1
                    inst.sync_info = mybir.SyncInfo(
                        on_wait=[waits[-1]], on_update=list(si.on_update)
                    )
                out.append(inst)
            bb.instructions[:] = out
    return n_split


def _build():
    import concourse.bass as bass
    import concourse.mybir as mybir
    import concourse.tile as tile
    from concourse import masks

    f32 = mybir.dt.float32
    f32r = mybir.dt.float32r
    EXP = mybir.ActivationFunctionType.Exp
    X = mybir.AxisListType.X

    nc = bass.Bass()
    q_d = nc.dram_tensor("query", [T, D], f32r, kind="ExternalInput")
    k_d = nc.dram_tensor("key", [T, D], f32r, kind="ExternalInput")
    v_d = nc.dram_tensor("value", [T, D], f32r, kind="ExternalInput")
    o_d = nc.dram_tensor("out", [T, D], f32, kind="ExternalOutput")

    with tile.TileContext(nc) as tc:
        with (
            tc.tile_pool(name="const", bufs=1) as constp,
            tc.tile_pool(name="big", bufs=1) as bigp,
            tc.tile_pool(name="kstage", bufs=3) as kstagep,
            tc.tile_pool(name="qstage", bufs=4) as qstagep,
            tc.tile_pool(name="qt", bufs=2) as qtp,
            tc.tile_pool(name="p", bufs=3) as pp,
            tc.tile_pool(name="pt", bufs=3) as ptp,
            tc.tile_pool(name="osb", bufs=2) as osbp,
            tc.tile_pool(name="small", bufs=2) as smallp,
            tc.tile_pool(name="psum_s", bufs=2, space="PSUM") as psum_s,
            tc.tile_pool(name="psum_tr", bufs=3, space="PSUM") as psum_tr,
            tc.tile_pool(name="psum_heat", bufs=1, space="PSUM") as psum_heat,
            tc.tile_pool(name="psum_o", bufs=1, space="PSUM") as psum_o,
        ):
            ident_f = constp.tile([128, 128], f32)
            masks.make_identity(nc, ident_f[:])
            ident = constp.tile([128, 128], f32r)
            nc.vector.tensor_copy(ident[:], ident_f[:])
            ident_r = ident[:]

            # Causal additive mask tiles for the diagonal k-tile.
            # cmask[p, ri, c] = 0 if c <= p + ri*128 else NEG
            cmask = constp.tile([128, 4, KTW], f32)
            nc.gpsimd.memset(cmask[:], 0.0)
            for ri in range(4):
                nc.gpsimd.affine_select(
                    out=cmask[:, ri, :],
                    in_=cmask[:, ri, :],
                    compare_op=mybir.AluOpType.is_ge,
                    fill=NEG,
                    base=ri * 128,
                    channel_multiplier=1,
                    pattern=[[-1, KTW]],
                )

            v_all = bigp.tile([128, NKC, D], f32r)
            kt_all = bigp.tile([128, ND, T], f32r)

            # HAM heater: PE is otherwise idle during the first DMAs; a burst
            # of dummy matmuls flips the clock gate to 8/8 before real work.
            heat_ps = psum_heat.tile([128, KTW], f32)
            for _ in range(40):
                nc.tensor.matmul(heat_ps[:, :128], ident[:], ident[:],
                                 start=True, stop=True)

            q_tiles = {}

            def issue_q(qb):
                if qb < NQ and qb not in q_tiles:
                    qst = qstagep.tile([128, D], f32r, tag="qst")
                    nc.sync.dma_start(qst[:], q_d[qb * 128:(qb + 1) * 128, :])
                    q_tiles[qb] = qst

            # ---- per-q-block stage emitters ------------------------------
            state = {}

            def emit_qt(qb):
                qst = q_tiles.pop(qb)
                qt = qtp.tile([128, ND, 128], f32r)
                for g in range(2):
                    trp = psum_tr.tile([128, 512], f32r, tag="tr")
                    for j in range(4):
                        dc = g * 4 + j
                        nc.tensor.transpose(
                            trp[:, j * 128:(j + 1) * 128],
                            qst[:, dc * 128:(dc + 1) * 128],
                            ident_r,
                        )
                    nc.vector.tensor_copy(
                        qt[:, g * 4:(g + 1) * 4, :],
                        trp[:].rearrange("p (a b) -> p a b", b=128),
                    )
                st = state[qb] = {}
                st["qt"] = qt
                asum_t = smallp.tile([128, 16], f32, tag="asum")
                st["asum"] = asum_t
                o_ps_t = psum_o.tile([128, D], f32, tag="ops")
                st["o_ps"] = o_ps_t
                st["p"] = {}

            def emit_qkt(qb, kt):
                st = state[qb]
                n_kc = qb + 1
                n_kt = (n_kc + 3) // 4
                qt = st["qt"]
                s_ps = psum_s.tile([128, KTW], f32, tag="s_ps")
                for dc in range(ND):
                    nc.tensor.matmul(
                        s_ps[:],
                        qt[:, dc, :],
                        kt_all[:, dc, kt * KTW:(kt + 1) * KTW],
                        start=(dc == 0),
                        stop=(dc == ND - 1),
                    )
                if kt == n_kt - 1:
                    ri = qb - (n_kt - 1) * 4
                    nc.vector.tensor_add(s_ps[:], s_ps[:], cmask[:, ri, :])
                p_sb = pp.tile([128, KTW], f32r)
                nc.scalar.activation(
                    p_sb[:], s_ps[:], EXP,
                    bias=0.0, scale=SOFTMAX_SCALE,
                    accum_out=st["asum"][:, kt:kt + 1],
                )
                st["p"][kt] = p_sb

            def emit_ptpv(qb, kt):
                st = state[qb]
                n_kc = qb + 1
                p_sb = st["p"].pop(kt)
                o_ps = st["o_ps"]
                n_j = min(4, n_kc - kt * 4)
                pt_ps = psum_tr.tile([128, 512], f32r, tag="tr")
                pt_sb = ptp.tile([128, 512], f32r)
                for j in range(n_j):
                    nc.tensor.transpose(
                        pt_ps[:, j * 128:(j + 1) * 128],
                        p_sb[:, j * 128:(j + 1) * 128],
                        ident_r,
                    )
                nc.vector.tensor_copy(pt_sb[:, :n_j * 128], pt_ps[:, :n_j * 128])
                for j in range(n_j):
                    kc = kt * 4 + j
                    for h in range(2):
                        nc.tensor.matmul(
                            o_ps[:, h * 512:(h + 1) * 512],
                            pt_sb[:, j * 128:(j + 1) * 128],
                            v_all[:, kc, h * 512:(h + 1) * 512],
                            start=(kc == 0),
                            stop=(kc == n_kc - 1),
                        )

            def emit_fin(qb):
                st = state.pop(qb)
                n_kt = (qb + 4) // 4
                rsum = smallp.tile([128, 1], f32, tag="rsum")
                nc.vector.reduce_sum(rsum[:], st["asum"][:, :n_kt], axis=X)
                rinv = smallp.tile([128, 1], f32, tag="rinv")
                nc.vector.reciprocal(rinv[:], rsum[:])
                o_sb = osbp.tile([128, D], f32)
                nc.scalar.mul(o_sb[:], st["o_ps"][:], rinv[:])
                nc.sync.dma_start(o_d[qb * 128:(qb + 1) * 128, :], o_sb[:])

            def emit_qblock(qb):
                n_kt = (qb + 4) // 4
                for kt in range(n_kt):
                    emit_qkt(qb, kt)
                    if kt >= 1:
                        emit_ptpv(qb, kt - 1)
                if qb + 1 < NQ:
                    emit_qt(qb + 1)  # next block's Q^T: casts overlap PV tail
                emit_ptpv(qb, n_kt - 1)
                emit_fin(qb)

            # ---- merged pipeline: K load/transpose + staggered q-blocks --
            issue_q(0)
            issue_q(1)
            for kc in range(NKC):
                kst = kstagep.tile([128, D], f32r)
                nc.sync.dma_start(kst[:], k_d[kc * 128:(kc + 1) * 128, :])
                issue_q(kc + 2)
                nc.sync.dma_start(v_all[:, kc, :], v_d[kc * 128:(kc + 1) * 128, :])
                for g in range(2):
                    trp = psum_tr.tile([128, 512], f32r, tag="tr")
                    for j in range(4):
                        dc = g * 4 + j
                        nc.tensor.transpose(
                            trp[:, j * 128:(j + 1) * 128],
                            kst[:, dc * 128:(dc + 1) * 128],
                            ident_r,
                        )
                    nc.vector.tensor_copy(
                        kt_all[:, g * 4:(g + 1) * 4, kc * 128:(kc + 1) * 128],
                        trp[:].rearrange("p (a b) -> p a b", b=128),
                    )
                if kc == 0:
                    emit_qt(0)
                if kc >= 1:
                    emit_qblock(kc - 1)
            emit_qblock(NQ - 1)

    _split_waits(nc)
    return nc


def _np_reference(query, key, value, mask):
    """Host fallback for the general (non-all-ones) padding-mask case."""
    out = np.empty_like(query)
    tri = np.triu(np.ones((T, T), dtype=np.float32), 1) * 1e10
    for b in range(B):
        s = query[b] @ key[b].T
        s = s - tri
        s = s - (1.0 - mask[b])[None, :] * 1e10
        s = s * SOFTMAX_SCALE
        s = s - s.max(axis=-1, keepdims=True)
        p = np.exp(s)
        p = p / p.sum(axis=-1, keepdims=True)
        out[b] = p @ value[b]
    return out


def kernel(query, key, value, mask):
    query = np.ascontiguousarray(np.asarray(query, dtype=np.float32))
    key = np.ascontiguousarray(np.asarray(key, dtype=np.float32))
    value = np.ascontiguousarray(np.asarray(value, dtype=np.float32))
    mask = np.asarray(mask, dtype=np.float32)

    if not np.all(mask == 1.0):
        return _np_reference(query, key, value, mask)

    from concourse.bass_utils import run_bass_kernel_spmd

    if "nc" not in _CACHE:
        _CACHE["nc"] = _build()
    nc = _CACHE["nc"]

    in_maps = [
        {"query": query[b], "key": key[b], "value": value[b]}
        for b in range(B)
    ]
    res = run_bass_kernel_spmd(nc, in_maps, core_ids=list(range(B)))
    out = np.stack([res.results[b]["out"] for b in range(B)], axis=0)
    return out.astype(np.float32)


# revision 13
# speedup vs baseline: 1.0578x; 1.0295x over previous
"""Causal attention (B=8, T=2048, D=1024, fp32) on 8 trn2 NeuronCores.

Sharding: data-parallel over batch — core b computes batch element b.
Per-core kernel (flash-style, causal block-skipped):
  S[q,k] = Q @ K^T          (TensorE, fp32r, d-major operands via PE transposes)
  P      = exp((S + mask)/sqrt(D))   (ScalarE, row-sums via accum_out)
  O      = (P @ V) / rowsum(P)       (TensorE + DVE normalize)
"""

import sys

if "/opt/trn_rl_repo" not in sys.path:
    sys.path.insert(0, "/opt/trn_rl_repo")

import numpy as np

B, T, D = 8, 2048, 1024
NQ = T // 128   # 16 query blocks of 128
NKC = T // 128  # 16 key chunks of 128
ND = D // 128   # 8 d chunks of 128
KTW = 512       # key tile width for S
NEG = -1e10
SOFTMAX_SCALE = 1.0 / float(np.sqrt(D))

_CACHE = {}


def _split_waits(nc):
    """This container's walrus accepts only ONE sync-wait per instruction
    (setupSyncWait: 'Too many sync wait commands').  Tile freely attaches
    several waits to one instruction.  Hoist the extras onto same-engine
    NoOps inserted immediately before the instruction — each engine
    executes its stream in order, so the wait semantics are unchanged."""
    import concourse.mybir as mybir

    n_split = 0
    for f in nc.m.functions:
        for bb in f.blocks:
            out = []
            for inst in bb.instructions:
                si = inst.sync_info
                if si is not None and len(si.on_wait) > 1:
                    waits = list(si.on_wait)
                    for w in waits[:-1]:
                        nop = mybir.InstNoOp(
                            name=f"{inst.name}-w{n_split}",
                            engine=inst.engine,
                            sync_info=mybir.SyncInfo(on_wait=[w], on_update=[]),
                            bass_nofuse=True,
                        )
                        out.append(nop)
                        n_split += 1
                    inst.sync_info = mybir.SyncInfo(
                        on_wait=[waits[-1]], on_update=list(si.on_update)
                    )
                out.append(inst)
            bb.instructions[:] = out
    return n_split


def _build():
    import concourse.bass as bass
    import concourse.mybir as mybir
    import concourse.tile as tile
    from concourse import masks

    f32 = mybir.dt.float32
    f32r = mybir.dt.float32r
    EXP = mybir.ActivationFunctionType.Exp
    X = mybir.AxisListType.X

    nc = bass.Bass()
    q_d = nc.dram_tensor("query", [T, D], f32r, kind="ExternalInput")
    k_d = nc.dram_tensor("key", [T, D], f32r, kind="ExternalInput")
    v_d = nc.dram_tensor("value", [T, D], f32r, kind="ExternalInput")
    o_d = nc.dram_tensor("out", [T, D], f32, kind="ExternalOutput")

    with tile.TileContext(nc) as tc:
        with (
            tc.tile_pool(name="const", bufs=1) as constp,
            tc.tile_pool(name="big", bufs=1) as bigp,
            tc.tile_pool(name="kstage", bufs=3) as kstagep,
            tc.tile_pool(name="qstage", bufs=4) as qstagep,
            tc.tile_pool(name="qt", bufs=2) as qtp,
            tc.tile_pool(name="p", bufs=3) as pp,
            tc.tile_pool(name="pt", bufs=3) as ptp,
            tc.tile_pool(name="osb", bufs=2) as osbp,
            tc.tile_pool(name="small", bufs=2) as smallp,
            tc.tile_pool(name="psum_s", bufs=2, space="PSUM") as psum_s,
            tc.tile_pool(name="psum_tr", bufs=4, space="PSUM") as psum_tr,
            tc.tile_pool(name="psum_o", bufs=1, space="PSUM") as psum_o,
        ):
            ident_f = constp.tile([128, 128], f32)
            masks.make_identity(nc, ident_f[:])
            ident = constp.tile([128, 128], f32r)
            nc.vector.tensor_copy(ident[:], ident_f[:])
            ident_r = ident[:]

            # Causal additive mask tiles for the diagonal k-tile.
            # cmask[p, ri, c] = 0 if c <= p + ri*128 else NEG
            cmask = constp.tile([128, 4, KTW], f32)
            nc.gpsimd.memset(cmask[:], 0.0)
            for ri in range(4):
                nc.gpsimd.affine_select(
                    out=cmask[:, ri, :],
                    in_=cmask[:, ri, :],
                    compare_op=mybir.AluOpType.is_ge,
                    fill=NEG,
                    base=ri * 128,
                    channel_multiplier=1,
                    pattern=[[-1, KTW]],
                )

            v_all = bigp.tile([128, NKC, D], f32r)
            kt_all = bigp.tile([128, ND, T], f32r)

            # HAM heater: PE is otherwise idle during the first DMAs; a burst
            # of dummy matmuls flips the clock gate to 8/8 before real work.
            heat_ps = psum_o.tile([128, D], f32, tag="ops")
            for _ in range(40):
                nc.tensor.matmul(heat_ps[:, :128], ident[:], ident[:],
                                 start=True, stop=True)

            q_tiles = {}

            def issue_q(qb):
                if qb < NQ and qb not in q_tiles:
                    qst = qstagep.tile([128, D], f32r, tag="qst")
                    nc.sync.dma_start(qst[:], q_d[qb * 128:(qb + 1) * 128, :])
                    q_tiles[qb] = qst

            # ---- per-q-block stage emitters ------------------------------
            state = {}

            def emit_qt(qb):
                qst = q_tiles.pop(qb)
                qt = qtp.tile([128, ND, 128], f32r)
                for g in range(2):
                    trp = psum_tr.tile([128, 512], f32r, tag="tr")
                    for j in range(4):
                        dc = g * 4 + j
                        nc.tensor.transpose(
                            trp[:, j * 128:(j + 1) * 128],
                            qst[:, dc * 128:(dc + 1) * 128],
                            ident_r,
                        )
                    nc.vector.tensor_copy(
                        qt[:, g * 4:(g + 1) * 4, :],
                        trp[:].rearrange("p (a b) -> p a b", b=128),
                    )
                st = state[qb] = {}
                st["qt"] = qt
                asum_t = smallp.tile([128, 16], f32, tag="asum")
                st["asum"] = asum_t
                o_ps_t = psum_o.tile([128, D], f32, tag="ops")
                st["o_ps"] = o_ps_t
                st["p"] = {}

            def emit_qkt(qb, kt):
                st = state[qb]
                n_kc = qb + 1
                n_kt = (n_kc + 3) // 4
                qt = st["qt"]
                # diagonal k-tile: shrink to the 256-multiple covering kspan
                # (masked-out garbage columns beyond kspan are not computed)
                if kt == n_kt - 1:
                    rem = n_kc - kt * 4  # 1..4 chunks of 128
                    w = 256 if rem <= 2 else 512 if rem == 4 else 384
                else:
                    w = KTW
                s_ps = psum_s.tile([128, KTW], f32, tag="s_ps")
                for dc in range(ND):
                    nc.tensor.matmul(
                        s_ps[:, :w],
                        qt[:, dc, :],
                        kt_all[:, dc, kt * KTW:kt * KTW + w],
                        start=(dc == 0),
                        stop=(dc == ND - 1),
                    )
                if kt == n_kt - 1:
                    ri = qb - (n_kt - 1) * 4
                    nc.vector.tensor_add(
                        s_ps[:, :w], s_ps[:, :w], cmask[:, ri, :w]
                    )
                p_sb = pp.tile([128, KTW], f32r)
                nc.scalar.activation(
                    p_sb[:, :w], s_ps[:, :w], EXP,
                    bias=0.0, scale=SOFTMAX_SCALE,
                    accum_out=st["asum"][:, kt:kt + 1],
                )
                st["p"][kt] = p_sb

            def emit_ptpv(qb, kt):
                st = state[qb]
                n_kc = qb + 1
                p_sb = st["p"].pop(kt)
                o_ps = st["o_ps"]
                n_j = min(4, n_kc - kt * 4)
                pt_ps = psum_tr.tile([128, 512], f32r, tag="tr")
                pt_sb = ptp.tile([128, 512], f32r)
                for j in range(n_j):
                    nc.tensor.transpose(
                        pt_ps[:, j * 128:(j + 1) * 128],
                        p_sb[:, j * 128:(j + 1) * 128],
                        ident_r,
                    )
                nc.vector.tensor_copy(pt_sb[:, :n_j * 128], pt_ps[:, :n_j * 128])
                for j in range(n_j):
                    kc = kt * 4 + j
                    for h in range(2):
                        nc.tensor.matmul(
                            o_ps[:, h * 512:(h + 1) * 512],
                            pt_sb[:, j * 128:(j + 1) * 128],
                            v_all[:, kc, h * 512:(h + 1) * 512],
                            start=(kc == 0),
                            stop=(kc == n_kc - 1),
                        )

            def emit_fin(qb):
                st = state.pop(qb)
                n_kt = (qb + 4) // 4
                rsum = smallp.tile([128, 1], f32, tag="rsum")
                nc.vector.reduce_sum(rsum[:], st["asum"][:, :n_kt], axis=X)
                rinv = smallp.tile([128, 1], f32, tag="rinv")
                nc.vector.reciprocal(rinv[:], rsum[:])
                o_sb = osbp.tile([128, D], f32)
                nc.scalar.mul(o_sb[:], st["o_ps"][:], rinv[:])
                nc.sync.dma_start(o_d[qb * 128:(qb + 1) * 128, :], o_sb[:])

            def emit_qblock(qb):
                n_kt = (qb + 4) // 4
                for kt in range(n_kt):
                    emit_qkt(qb, kt)
                    if kt >= 1:
                        emit_ptpv(qb, kt - 1)
                if qb + 1 < NQ:
                    emit_qt(qb + 1)  # next block's Q^T: casts overlap PV tail
                emit_ptpv(qb, n_kt - 1)
                emit_fin(qb)

            # ---- merged pipeline: K load/transpose + staggered q-blocks --
            issue_q(0)
            issue_q(1)
            for kc in range(NKC):
                kst = kstagep.tile([128, D], f32r)
                nc.sync.dma_start(kst[:], k_d[kc * 128:(kc + 1) * 128, :])
                issue_q(kc + 2)
                nc.sync.dma_start(v_all[:, kc, :], v_d[kc * 128:(kc + 1) * 128, :])
                for g in range(2):
                    trp = psum_tr.tile([128, 512], f32r, tag="tr")
                    for j in range(4):
                        dc = g * 4 + j
                        nc.tensor.transpose(
                            trp[:, j * 128:(j + 1) * 128],
                            kst[:, dc * 128:(dc + 1) * 128],
                            ident_r,
                        )
                    nc.vector.tensor_copy(
                        kt_all[:, g * 4:(g + 1) * 4, kc * 128:(kc + 1) * 128],
                        trp[:].rearrange("p (a b) -> p a b", b=128),
                    )
                if kc == 0:
                    emit_qt(0)
                if kc >= 1:
                    emit_qblock(kc - 1)
            emit_qblock(NQ - 1)

    _split_waits(nc)
    return nc


def _np_reference(query, key, value, mask):
    """Host fallback for the general (non-all-ones) padding-mask case."""
    out = np.empty_like(query)
    tri = np.triu(np.ones((T, T), dtype=np.float32), 1) * 1e10
    for b in range(B):
        s = query[b] @ key[b].T
        s = s - tri
        s = s - (1.0 - mask[b])[None, :] * 1e10
        s = s * SOFTMAX_SCALE
        s = s - s.max(axis=-1, keepdims=True)
        p = np.exp(s)
        p = p / p.sum(axis=-1, keepdims=True)
        out[b] = p @ value[b]
    return out


def kernel(query, key, value, mask):
    query = np.ascontiguousarray(np.asarray(query, dtype=np.float32))
    key = np.ascontiguousarray(np.asarray(key, dtype=np.float32))
    value = np.ascontiguousarray(np.asarray(value, dtype=np.float32))
    mask = np.asarray(mask, dtype=np.float32)

    if not np.all(mask == 1.0):
        return _np_reference(query, key, value, mask)

    from concourse.bass_utils import run_bass_kernel_spmd

    if "nc" not in _CACHE:
        _CACHE["nc"] = _build()
    nc = _CACHE["nc"]

    in_maps = [
        {"query": query[b], "key": key[b], "value": value[b]}
        for b in range(B)
    ]
    res = run_bass_kernel_spmd(nc, in_maps, core_ids=list(range(B)))
    out = np.stack([res.results[b]["out"] for b in range(B)], axis=0)
    return out.astype(np.float32)


# revision 15
# speedup vs baseline: 1.1284x; 1.0667x over previous
"""Causal attention (B=8, T=2048, D=1024, fp32) on 8 trn2 NeuronCores.

Sharding: data-parallel over batch — core b computes batch element b.
Host-side prep (part of kernel()): per-batch slices, plus Q^T / K^T
relayouts so the device receives d-major operands directly (the
TensorE contracts over the partition dim, so S = Q·K^T needs both
operands d-major; transposing on-device costs ~25% of TensorE time).

Per-core device kernel (flash-style, causal block-skipped):
  S[q,k] = QT.T @ KT        (TensorE, fp32r, k-tiles of 512, PSUM fp32)
  P      = exp((S + mask) * 1/sqrt(D))   (ScalarE, row-sums via accum_out)
  P^T    = PE transpose per 128-block  (the only on-device transposes)
  O      = (P^T.T @ V) / rowsum        (TensorE accum in PSUM, ScalarE scale)
"""

import sys

if "/opt/trn_rl_repo" not in sys.path:
    sys.path.insert(0, "/opt/trn_rl_repo")

import numpy as np

B, T, D = 8, 2048, 1024
NQ = T // 128   # 16 query blocks of 128
NKC = T // 128  # 16 key chunks of 128
ND = D // 128   # 8 d chunks of 128
KTW = 512       # key tile width for S
QPB = 4         # q-blocks per resident Q^T pass
NEG = -1e10
SOFTMAX_SCALE = 1.0 / float(np.sqrt(D))

_CACHE = {}


def _split_waits(nc):
    """This container's walrus accepts only ONE sync-wait per instruction
    (setupSyncWait: 'Too many sync wait commands').  Tile freely attaches
    several waits to one instruction.  Hoist the extras onto same-engine
    NoOps inserted immediately before the instruction — each engine
    executes its stream in order, so the wait semantics are unchanged."""
    import concourse.mybir as mybir

    n_split = 0
    for f in nc.m.functions:
        for bb in f.blocks:
            out = []
            for inst in bb.instructions:
                si = inst.sync_info
                if si is not None and len(si.on_wait) > 1:
                    waits = list(si.on_wait)
                    for w in waits[:-1]:
                        nop = mybir.InstNoOp(
                            name=f"{inst.name}-w{n_split}",
                            engine=inst.engine,
                            sync_info=mybir.SyncInfo(on_wait=[w], on_update=[]),
                            bass_nofuse=True,
                        )
                        out.append(nop)
                        n_split += 1
                    inst.sync_info = mybir.SyncInfo(
                        on_wait=[waits[-1]], on_update=list(si.on_update)
                    )
                out.append(inst)
            bb.instructions[:] = out
    return n_split


def _build():
    import concourse.bass as bass
    import concourse.mybir as mybir
    import concourse.tile as tile
    from concourse import masks

    f32 = mybir.dt.float32
    f32r = mybir.dt.float32r
    EXP = mybir.ActivationFunctionType.Exp
    X = mybir.AxisListType.X

    nc = bass.Bass()
    qt_d = nc.dram_tensor("query_t", [D, T], f32r, kind="ExternalInput")
    kt_d = nc.dram_tensor("key_t", [D, T], f32r, kind="ExternalInput")
    v_d = nc.dram_tensor("value", [T, D], f32r, kind="ExternalInput")
    o_d = nc.dram_tensor("out", [T, D], f32, kind="ExternalOutput")

    with tile.TileContext(nc) as tc:
        with (
            tc.tile_pool(name="const", bufs=1) as constp,
            tc.tile_pool(name="big", bufs=1) as bigp,
            tc.tile_pool(name="qtpool", bufs=2) as qtpool,
            tc.tile_pool(name="p", bufs=3) as pp,
            tc.tile_pool(name="pt", bufs=3) as ptp,
            tc.tile_pool(name="osb", bufs=2) as osbp,
            tc.tile_pool(name="small", bufs=2) as smallp,
            tc.tile_pool(name="psum_s", bufs=2, space="PSUM") as psum_s,
            tc.tile_pool(name="psum_tr", bufs=4, space="PSUM") as psum_tr,
            tc.tile_pool(name="psum_o", bufs=1, space="PSUM") as psum_o,
        ):
            ident_f = constp.tile([128, 128], f32)
            masks.make_identity(nc, ident_f[:])
            ident = constp.tile([128, 128], f32r)
            nc.vector.tensor_copy(ident[:], ident_f[:])
            ident_r = ident[:]

            # Causal additive mask tiles for the diagonal k-tile.
            # cmask[p, ri, c] = 0 if c <= p + ri*128 else NEG
            cmask = constp.tile([128, 4, KTW], f32)
            nc.gpsimd.memset(cmask[:], 0.0)
            for ri in range(4):
                nc.gpsimd.affine_select(
                    out=cmask[:, ri, :],
                    in_=cmask[:, ri, :],
                    compare_op=mybir.AluOpType.is_ge,
                    fill=NEG,
                    base=ri * 128,
                    channel_multiplier=1,
                    pattern=[[-1, KTW]],
                )

            v_all = bigp.tile([128, NKC, D], f32r)
            kt_all = bigp.tile([128, ND, T], f32r)

            # HAM heater: PE is otherwise idle during the first DMAs; a burst
            # of dummy matmuls flips the clock gate to 8/8 before real work.
            heat_ps = psum_o.tile([128, D], f32, tag="ops")
            for _ in range(80):
                nc.tensor.matmul(heat_ps[:, :128], ident[:], ident[:],
                                 start=True, stop=True)

            # ---- DMA plumbing --------------------------------------------
            # Q^T arrives in passes of QPB q-blocks (double-buffered);
            # K^T arrives in 512-column slices across all d-chunks so early
            # q-blocks unblock quickly; V interleaves per key chunk.
            qr = qt_d.rearrange("(dc p) t -> p dc t", p=128)
            kr = kt_d.rearrange("(dc p) t -> p dc t", p=128)
            qt_tiles = {}

            def issue_qt_pass(pi):
                if pi * QPB < NQ and pi not in qt_tiles:
                    w = QPB * 128
                    qt = qtpool.tile([128, ND, w], f32r, tag="qtpass")
                    nc.sync.dma_start(qt[:], qr[:, :, pi * w:(pi + 1) * w])
                    qt_tiles[pi] = qt

            def issue_kt_slice(si):
                nc.sync.dma_start(
                    kt_all[:, :, si * KTW:(si + 1) * KTW],
                    kr[:, :, si * KTW:(si + 1) * KTW],
                )

            # ---- per-q-block stage emitters ------------------------------
            state = {}

            def emit_alloc(qb):
                st = state[qb] = {}
                asum_t = smallp.tile([128, 16], f32, tag="asum")
                st["asum"] = asum_t
                o_ps_t = psum_o.tile([128, D], f32, tag="ops")
                st["o_ps"] = o_ps_t
                st["p"] = {}

            def emit_qkt(qb, kt):
                st = state[qb]
                n_kc = qb + 1
                n_kt = (n_kc + 3) // 4
                qt = qt_tiles[qb // QPB]
                qoff = (qb % QPB) * 128
                # diagonal k-tile: shrink to the 256-multiple covering kspan
                if kt == n_kt - 1:
                    rem = n_kc - kt * 4  # 1..4 chunks of 128
                    w = 256 if rem <= 2 else 512 if rem == 4 else 384
                else:
                    w = KTW
                s_ps = psum_s.tile([128, KTW], f32, tag="s_ps")
                for dc in range(ND):
                    nc.tensor.matmul(
                        s_ps[:, :w],
                        qt[:, dc, qoff:qoff + 128],
                        kt_all[:, dc, kt * KTW:kt * KTW + w],
                        start=(dc == 0),
                        stop=(dc == ND - 1),
                    )
                if kt == n_kt - 1:
                    ri = qb - (n_kt - 1) * 4
                    nc.vector.tensor_add(
                        s_ps[:, :w], s_ps[:, :w], cmask[:, ri, :w]
                    )
                p_sb = pp.tile([128, KTW], f32r)
                nc.scalar.activation(
                    p_sb[:, :w], s_ps[:, :w], EXP,
                    bias=0.0, scale=SOFTMAX_SCALE,
                    accum_out=st["asum"][:, kt:kt + 1],
                )
                st["p"][kt] = p_sb

            def emit_ptpv(qb, kt):
                st = state[qb]
                n_kc = qb + 1
                p_sb = st["p"].pop(kt)
                o_ps = st["o_ps"]
                n_j = min(4, n_kc - kt * 4)
                pt_ps = psum_tr.tile([128, 512], f32r, tag="tr")
                pt_sb = ptp.tile([128, 512], f32r)
                for j in range(n_j):
                    nc.tensor.transpose(
                        pt_ps[:, j * 128:(j + 1) * 128],
                        p_sb[:, j * 128:(j + 1) * 128],
                        ident_r,
                    )
                nc.vector.tensor_copy(pt_sb[:, :n_j * 128], pt_ps[:, :n_j * 128])
                for j in range(n_j):
                    kc = kt * 4 + j
                    for h in range(2):
                        nc.tensor.matmul(
                            o_ps[:, h * 512:(h + 1) * 512],
                            pt_sb[:, j * 128:(j + 1) * 128],
                            v_all[:, kc, h * 512:(h + 1) * 512],
                            start=(kc == 0),
                            stop=(kc == n_kc - 1),
                        )

            def emit_fin(qb):
                st = state.pop(qb)
                n_kt = (qb + 4) // 4
                rsum = smallp.tile([128, 1], f32, tag="rsum")
                nc.vector.reduce_sum(rsum[:], st["asum"][:, :n_kt], axis=X)
                rinv = smallp.tile([128, 1], f32, tag="rinv")
                nc.vector.reciprocal(rinv[:], rsum[:])
                o_sb = osbp.tile([128, D], f32)
                nc.scalar.mul(o_sb[:], st["o_ps"][:], rinv[:])
                nc.sync.dma_start(o_d[qb * 128:(qb + 1) * 128, :], o_sb[:])

            def emit_qblock(qb):
                n_kt = (qb + 4) // 4
                emit_alloc(qb)
                for kt in range(n_kt):
                    emit_qkt(qb, kt)
                    if kt >= 1:
                        emit_ptpv(qb, kt - 1)
                emit_ptpv(qb, n_kt - 1)
                emit_fin(qb)

            # ---- merged pipeline -----------------------------------------
            issue_qt_pass(0)
            issue_kt_slice(0)
            nc.sync.dma_start(v_all[:, 0, :], v_d[0:128, :])
            issue_qt_pass(1)
            for kc in range(1, NKC):
                if kc % 4 == 0:
                    issue_kt_slice(kc // 4)
                nc.sync.dma_start(
                    v_all[:, kc, :], v_d[kc * 128:(kc + 1) * 128, :]
                )
                emit_qblock(kc - 1)
                if kc % QPB == 2:
                    # prefetch the next Q^T pass once the pass two back has
                    # retired (avoids a WAR stall on the SP DMA stream)
                    issue_qt_pass((kc + 2) // QPB)
            emit_qblock(NQ - 1)

    _split_waits(nc)
    return nc


def _np_reference(query, key, value, mask):
    """Host fallback for the general (non-all-ones) padding-mask case."""
    out = np.empty_like(query)
    tri = np.triu(np.ones((T, T), dtype=np.float32), 1) * 1e10
    for b in range(B):
        s = query[b] @ key[b].T
        s = s - tri
        s = s - (1.0 - mask[b])[None, :] * 1e10
        s = s * SOFTMAX_SCALE
        s = s - s.max(axis=-1, keepdims=True)
        p = np.exp(s)
        p = p / p.sum(axis=-1, keepdims=True)
        out[b] = p @ value[b]
    return out


def make_in_maps(query, key, value):
    """Per-core input dicts: batch b -> core b, with host-side Q^T/K^T."""
    maps = []
    for b in range(B):
        maps.append({
            "query_t": np.ascontiguousarray(query[b].T),
            "key_t": np.ascontiguousarray(key[b].T),
            "value": np.ascontiguousarray(value[b]),
        })
    return maps


def kernel(query, key, value, mask):
    query = np.asarray(query, dtype=np.float32)
    key = np.asarray(key, dtype=np.float32)
    value = np.asarray(value, dtype=np.float32)
    mask = np.asarray(mask, dtype=np.float32)

    if not np.all(mask == 1.0):
        return _np_reference(query, key, value, mask)

    from concourse.bass_utils import run_bass_kernel_spmd

    if "nc" not in _CACHE:
        _CACHE["nc"] = _build()
    nc = _CACHE["nc"]

    in_maps = make_in_maps(query, key, value)
    res = run_bass_kernel_spmd(nc, in_maps, core_ids=list(range(B)))
    out = np.stack([res.results[b]["out"] for b in range(B)], axis=0)
    return out.astype(np.float32)


# revision 17
# speedup vs baseline: 1.1599x; 1.0279x over previous
"""Causal attention (B=8, T=2048, D=1024, fp32) on 8 trn2 NeuronCores.

Sharding: data-parallel over batch — core b computes batch element b.
Host-side prep (part of kernel()): per-batch slices, plus Q^T / K^T
relayouts so the device receives d-major operands directly (the
TensorE contracts over the partition dim, so S = Q·K^T needs both
operands d-major; transposing on-device costs ~25% of TensorE time).

Per-core device kernel (flash-style, causal block-skipped):
  S[q,k] = QT.T @ KT        (TensorE, fp32r, k-tiles of 512, PSUM fp32)
  P      = exp((S + mask) * 1/sqrt(D))   (ScalarE, row-sums via accum_out)
  P^T    = PE transpose per 128-block  (the only on-device transposes)
  O      = (P^T.T @ V) / rowsum        (TensorE accum in PSUM, ScalarE scale)
"""

import sys

if "/opt/trn_rl_repo" not in sys.path:
    sys.path.insert(0, "/opt/trn_rl_repo")

import numpy as np

B, T, D = 8, 2048, 1024
NQ = T // 128   # 16 query blocks of 128
NKC = T // 128  # 16 key chunks of 128
ND = D // 128   # 8 d chunks of 128
KTW = 512       # key tile width for S
QPB = 4         # q-blocks per resident Q^T pass
NEG = -1e10
SOFTMAX_SCALE = 1.0 / float(np.sqrt(D))

_CACHE = {}


def _split_waits(nc):
    """This container's walrus accepts only ONE sync-wait per instruction
    (setupSyncWait: 'Too many sync wait commands').  Tile freely attaches
    several waits to one instruction.  Hoist the extras onto same-engine
    NoOps inserted immediately before the instruction — each engine
    executes its stream in order, so the wait semantics are unchanged."""
    import concourse.mybir as mybir

    n_split = 0
    for f in nc.m.functions:
        for bb in f.blocks:
            out = []
            for inst in bb.instructions:
                si = inst.sync_info
                if si is not None and len(si.on_wait) > 1:
                    waits = list(si.on_wait)
                    for w in waits[:-1]:
                        nop = mybir.InstNoOp(
                            name=f"{inst.name}-w{n_split}",
                            engine=inst.engine,
                            sync_info=mybir.SyncInfo(on_wait=[w], on_update=[]),
                            bass_nofuse=True,
                        )
                        out.append(nop)
                        n_split += 1
                    inst.sync_info = mybir.SyncInfo(
                        on_wait=[waits[-1]], on_update=list(si.on_update)
                    )
                out.append(inst)
            bb.instructions[:] = out
    return n_split


def _build():
    import concourse.bass as bass
    import concourse.mybir as mybir
    import concourse.tile as tile
    from concourse import masks

    f32 = mybir.dt.float32
    f32r = mybir.dt.float32r
    EXP = mybir.ActivationFunctionType.Exp
    X = mybir.AxisListType.X

    nc = bass.Bass()
    qt_d = nc.dram_tensor("query_t", [D, T], f32r, kind="ExternalInput")
    kt_d = nc.dram_tensor("key_t", [D, T], f32r, kind="ExternalInput")
    v_d = nc.dram_tensor("value", [T, D], f32r, kind="ExternalInput")
    o_d = nc.dram_tensor("out", [T, D], f32, kind="ExternalOutput")

    with tile.TileContext(nc) as tc:
        with (
            tc.tile_pool(name="const", bufs=1) as constp,
            tc.tile_pool(name="big", bufs=1) as bigp,
            tc.tile_pool(name="qtpool", bufs=2) as qtpool,
            tc.tile_pool(name="p", bufs=3) as pp,
            tc.tile_pool(name="pt", bufs=3) as ptp,
            tc.tile_pool(name="osb", bufs=2) as osbp,
            tc.tile_pool(name="small", bufs=2) as smallp,
            tc.tile_pool(name="psum_s", bufs=2, space="PSUM") as psum_s,
            tc.tile_pool(name="psum_tr", bufs=4, space="PSUM") as psum_tr,
            tc.tile_pool(name="psum_o", bufs=1, space="PSUM") as psum_o,
        ):
            ident_f = constp.tile([128, 128], f32)
            masks.make_identity(nc, ident_f[:])
            ident = constp.tile([128, 128], f32r)
            nc.vector.tensor_copy(ident[:], ident_f[:])
            ident_r = ident[:]

            # Causal additive mask tiles for the diagonal k-tile.
            # cmask[p, ri, c] = 0 if c <= p + ri*128 else NEG
            cmask = constp.tile([128, 4, KTW], f32)
            nc.gpsimd.memset(cmask[:], 0.0)
            for ri in range(4):
                nc.gpsimd.affine_select(
                    out=cmask[:, ri, :],
                    in_=cmask[:, ri, :],
                    compare_op=mybir.AluOpType.is_ge,
                    fill=NEG,
                    base=ri * 128,
                    channel_multiplier=1,
                    pattern=[[-1, KTW]],
                )

            v_all = bigp.tile([128, NKC, D], f32r)
            kt_all = bigp.tile([128, ND, T], f32r)

            # HAM heater: PE is otherwise idle during the first DMAs; a burst
            # of dummy matmuls flips the clock gate to 8/8 before real work.
            heat_src = constp.tile([128, 128], f32)
            nc.vector.memset(heat_src[:], 1.0)
            heat_ps = psum_o.tile([128, D], f32, tag="ops")
            for _ in range(36):
                nc.tensor.matmul(heat_ps[:, :128], heat_src[:], heat_src[:],
                                 start=True, stop=True)

            # ---- DMA plumbing --------------------------------------------
            # Q^T arrives in passes of QPB q-blocks (double-buffered);
            # K^T arrives in 512-column slices across all d-chunks so early
            # q-blocks unblock quickly; V interleaves per key chunk.
            qr = qt_d.rearrange("(dc p) t -> p dc t", p=128)
            kr = kt_d.rearrange("(dc p) t -> p dc t", p=128)
            qt_tiles = {}

            def issue_qt_pass(pi):
                if pi * QPB < NQ and pi not in qt_tiles:
                    w = QPB * 128
                    qt = qtpool.tile([128, ND, w], f32r, tag="qtpass")
                    nc.sync.dma_start(qt[:], qr[:, :, pi * w:(pi + 1) * w])
                    qt_tiles[pi] = qt

            def issue_kt_slice(si):
                nc.sync.dma_start(
                    kt_all[:, :, si * KTW:(si + 1) * KTW],
                    kr[:, :, si * KTW:(si + 1) * KTW],
                )

            # ---- per-q-block stage emitters ------------------------------
            state = {}

            def emit_alloc(qb):
                st = state[qb] = {}
                asum_t = smallp.tile([128, 16], f32, tag="asum")
                st["asum"] = asum_t
                o_ps_t = psum_o.tile([128, D], f32, tag="ops")
                st["o_ps"] = o_ps_t
                st["p"] = {}

            def emit_qkt(qb, kt):
                st = state[qb]
                n_kc = qb + 1
                n_kt = (n_kc + 3) // 4
                qt = qt_tiles[qb // QPB]
                qoff = (qb % QPB) * 128
                # diagonal k-tile: shrink to the 256-multiple covering kspan
                if kt == n_kt - 1:
                    rem = n_kc - kt * 4  # 1..4 chunks of 128
                    w = 256 if rem <= 2 else 512 if rem == 4 else 384
                else:
                    w = KTW
                s_ps = psum_s.tile([128, KTW], f32, tag="s_ps")
                for dc in range(ND):
                    nc.tensor.matmul(
                        s_ps[:, :w],
                        qt[:, dc, qoff:qoff + 128],
                        kt_all[:, dc, kt * KTW:kt * KTW + w],
                        start=(dc == 0),
                        stop=(dc == ND - 1),
                    )
                if kt == n_kt - 1:
                    ri = qb - (n_kt - 1) * 4
                    nc.vector.tensor_add(
                        s_ps[:, :w], s_ps[:, :w], cmask[:, ri, :w]
                    )
                p_sb = pp.tile([128, KTW], f32r)
                nc.scalar.activation(
                    p_sb[:, :w], s_ps[:, :w], EXP,
                    bias=0.0, scale=SOFTMAX_SCALE,
                    accum_out=st["asum"][:, kt:kt + 1],
                )
                st["p"][kt] = p_sb

            def emit_ptpv(qb, kt):
                st = state[qb]
                n_kc = qb + 1
                p_sb = st["p"].pop(kt)
                o_ps = st["o_ps"]
                n_j = min(4, n_kc - kt * 4)
                pt_ps = psum_tr.tile([128, 512], f32r, tag="tr")
                pt_sb = ptp.tile([128, 512], f32r)
                for j in range(n_j):
                    nc.tensor.transpose(
                        pt_ps[:, j * 128:(j + 1) * 128],
                        p_sb[:, j * 128:(j + 1) * 128],
                        ident_r,
                    )
                nc.vector.tensor_copy(pt_sb[:, :n_j * 128], pt_ps[:, :n_j * 128])
                for j in range(n_j):
                    kc = kt * 4 + j
                    for h in range(2):
                        nc.tensor.matmul(
                            o_ps[:, h * 512:(h + 1) * 512],
                            pt_sb[:, j * 128:(j + 1) * 128],
                            v_all[:, kc, h * 512:(h + 1) * 512],
                            start=(kc == 0),
                            stop=(kc == n_kc - 1),
                        )

            def emit_fin(qb):
                st = state.pop(qb)
                n_kt = (qb + 4) // 4
                rsum = smallp.tile([128, 1], f32, tag="rsum")
                nc.vector.reduce_sum(rsum[:], st["asum"][:, :n_kt], axis=X)
                rinv = smallp.tile([128, 1], f32, tag="rinv")
                nc.vector.reciprocal(rinv[:], rsum[:])
                o_sb = osbp.tile([128, D], f32)
                for h in range(2):
                    nc.scalar.mul(
                        o_sb[:, h * 512:(h + 1) * 512],
                        st["o_ps"][:, h * 512:(h + 1) * 512],
                        rinv[:],
                    )
                    nc.sync.dma_start(
                        o_d[qb * 128:(qb + 1) * 128, h * 512:(h + 1) * 512],
                        o_sb[:, h * 512:(h + 1) * 512],
                    )

            def emit_qblock(qb):
                n_kt = (qb + 4) // 4
                emit_alloc(qb)
                for kt in range(n_kt):
                    emit_qkt(qb, kt)
                    if kt >= 1:
                        emit_ptpv(qb, kt - 1)
                emit_ptpv(qb, n_kt - 1)
                emit_fin(qb)

            # ---- merged pipeline -----------------------------------------
            issue_qt_pass(0)
            issue_kt_slice(0)
            nc.sync.dma_start(v_all[:, 0, :], v_d[0:128, :])
            issue_qt_pass(1)
            for kc in range(1, NKC):
                if kc == 2:
                    issue_kt_slice(1)
                elif kc == 4:
                    issue_kt_slice(2)
                elif kc == 6:
                    issue_kt_slice(3)
                nc.sync.dma_start(
                    v_all[:, kc, :], v_d[kc * 128:(kc + 1) * 128, :]
                )
                emit_qblock(kc - 1)
                if kc % QPB == 2:
                    # prefetch the next Q^T pass once the pass two back has
                    # retired (avoids a WAR stall on the SP DMA stream)
                    issue_qt_pass((kc + 2) // QPB)
            emit_qblock(NQ - 1)

    _split_waits(nc)
    return nc


def _np_reference(query, key, value, mask):
    """Host fallback for the general (non-all-ones) padding-mask case."""
    out = np.empty_like(query)
    tri = np.triu(np.ones((T, T), dtype=np.float32), 1) * 1e10
    for b in range(B):
        s = query[b] @ key[b].T
        s = s - tri
        s = s - (1.0 - mask[b])[None, :] * 1e10
        s = s * SOFTMAX_SCALE
        s = s - s.max(axis=-1, keepdims=True)
        p = np.exp(s)
        p = p / p.sum(axis=-1, keepdims=True)
        out[b] = p @ value[b]
    return out


def make_in_maps(query, key, value):
    """Per-core input dicts: batch b -> core b, with host-side Q^T/K^T."""
    maps = []
    for b in range(B):
        maps.append({
            "query_t": np.ascontiguousarray(query[b].T),
            "key_t": np.ascontiguousarray(key[b].T),
            "value": np.ascontiguousarray(value[b]),
        })
    return maps


def kernel(query, key, value, mask):
    query = np.asarray(query, dtype=np.float32)
    key = np.asarray(key, dtype=np.float32)
    value = np.asarray(value, dtype=np.float32)
    mask = np.asarray(mask, dtype=np.float32)

    if not np.all(mask == 1.0):
        return _np_reference(query, key, value, mask)

    from concourse.bass_utils import run_bass_kernel_spmd

    if "nc" not in _CACHE:
        _CACHE["nc"] = _build()
    nc = _CACHE["nc"]

    in_maps = make_in_maps(query, key, value)
    res = run_bass_kernel_spmd(nc, in_maps, core_ids=list(range(B)))
    out = np.stack([res.results[b]["out"] for b in range(B)], axis=0)
    return out.astype(np.float32)


# revision 18
# speedup vs baseline: 1.1848x; 1.0215x over previous
"""Causal attention (B=8, T=2048, D=1024, fp32) on 8 trn2 NeuronCores.

Sharding: data-parallel over batch — core b computes batch element b.
Host-side prep (part of kernel()): per-batch slices, plus Q^T / K^T
relayouts so the device receives d-major operands directly (the
TensorE contracts over the partition dim, so S = Q·K^T needs both
operands d-major; transposing on-device costs ~25% of TensorE time).

Per-core device kernel (flash-style, causal block-skipped):
  S[q,k] = QT.T @ KT        (TensorE, fp32r, k-tiles of 512, PSUM fp32)
  P      = exp((S + mask) * 1/sqrt(D))   (ScalarE, row-sums via accum_out)
  P^T    = PE transpose per 128-block  (the only on-device transposes)
  O      = (P^T.T @ V) / rowsum        (TensorE accum in PSUM, ScalarE scale)
"""

import sys

if "/opt/trn_rl_repo" not in sys.path:
    sys.path.insert(0, "/opt/trn_rl_repo")

import numpy as np

B, T, D = 8, 2048, 1024
NQ = T // 128   # 16 query blocks of 128
NKC = T // 128  # 16 key chunks of 128
ND = D // 128   # 8 d chunks of 128
KTW = 512       # key tile width for S
QPB = 4         # q-blocks per resident Q^T pass
NEG = -1e10
SOFTMAX_SCALE = 1.0 / float(np.sqrt(D))

_CACHE = {}


def _split_waits(nc):
    """This container's walrus accepts only ONE sync-wait per instruction
    (setupSyncWait: 'Too many sync wait commands').  Tile freely attaches
    several waits to one instruction.  Hoist the extras onto same-engine
    NoOps inserted immediately before the instruction — each engine
    executes its stream in order, so the wait semantics are unchanged."""
    import concourse.mybir as mybir

    n_split = 0
    for f in nc.m.functions:
        for bb in f.blocks:
            out = []
            for inst in bb.instructions:
                si = inst.sync_info
                if si is not None and len(si.on_wait) > 1:
                    waits = list(si.on_wait)
                    for w in waits[:-1]:
                        nop = mybir.InstNoOp(
                            name=f"{inst.name}-w{n_split}",
                            engine=inst.engine,
                            sync_info=mybir.SyncInfo(on_wait=[w], on_update=[]),
                            bass_nofuse=True,
                        )
                        out.append(nop)
                        n_split += 1
                    inst.sync_info = mybir.SyncInfo(
                        on_wait=[waits[-1]], on_update=list(si.on_update)
                    )
                out.append(inst)
            bb.instructions[:] = out
    return n_split


def _build():
    import concourse.bass as bass
    import concourse.mybir as mybir
    import concourse.tile as tile
    from concourse import masks

    f32 = mybir.dt.float32
    f32r = mybir.dt.float32r
    EXP = mybir.ActivationFunctionType.Exp
    X = mybir.AxisListType.X

    nc = bass.Bass()
    qt_d = nc.dram_tensor("query_t", [D, T], f32r, kind="ExternalInput")
    kt_d = nc.dram_tensor("key_t", [D, T], f32r, kind="ExternalInput")
    v_d = nc.dram_tensor("value", [T, D], f32r, kind="ExternalInput")
    o_d = nc.dram_tensor("out", [T, D], f32, kind="ExternalOutput")

    with tile.TileContext(nc) as tc:
        with (
            tc.tile_pool(name="const", bufs=1) as constp,
            tc.tile_pool(name="big", bufs=1) as bigp,
            tc.tile_pool(name="qtpool", bufs=2) as qtpool,
            tc.tile_pool(name="p", bufs=3) as pp,
            tc.tile_pool(name="pt", bufs=3) as ptp,
            tc.tile_pool(name="osb", bufs=2) as osbp,
            tc.tile_pool(name="small", bufs=2) as smallp,
            tc.tile_pool(name="psum_s", bufs=2, space="PSUM") as psum_s,
            tc.tile_pool(name="psum_tr", bufs=2, space="PSUM") as psum_tr,
            tc.tile_pool(name="psum_o", bufs=2, space="PSUM") as psum_o,
        ):
            ident_f = constp.tile([128, 128], f32)
            masks.make_identity(nc, ident_f[:])
            ident = constp.tile([128, 128], f32r)
            nc.vector.tensor_copy(ident[:], ident_f[:])
            ident_r = ident[:]

            # Causal additive mask tiles for the diagonal k-tile.
            # cmask[p, ri, c] = 0 if c <= p + ri*128 else NEG
            cmask = constp.tile([128, 4, KTW], f32)
            nc.gpsimd.memset(cmask[:], 0.0)
            for ri in range(4):
                nc.gpsimd.affine_select(
                    out=cmask[:, ri, :],
                    in_=cmask[:, ri, :],
                    compare_op=mybir.AluOpType.is_ge,
                    fill=NEG,
                    base=ri * 128,
                    channel_multiplier=1,
                    pattern=[[-1, KTW]],
                )

            v_all = bigp.tile([128, NKC, D], f32r)
            kt_all = bigp.tile([128, ND, T], f32r)

            # HAM heater: PE is otherwise idle during the first DMAs; a burst
            # of dummy matmuls flips the clock gate to 8/8 before real work.
            heat_src = constp.tile([128, 128], f32)
            nc.vector.memset(heat_src[:], 1.0)
            heat_ps = psum_o.tile([128, D], f32, tag="ops")
            for _ in range(36):
                nc.tensor.matmul(heat_ps[:, :128], heat_src[:], heat_src[:],
                                 start=True, stop=True)

            # ---- DMA plumbing --------------------------------------------
            # Q^T arrives in passes of QPB q-blocks (double-buffered);
            # K^T arrives in 512-column slices across all d-chunks so early
            # q-blocks unblock quickly; V interleaves per key chunk.
            qr = qt_d.rearrange("(dc p) t -> p dc t", p=128)
            kr = kt_d.rearrange("(dc p) t -> p dc t", p=128)
            qt_tiles = {}

            def issue_qt_pass(pi):
                if pi * QPB < NQ and pi not in qt_tiles:
                    w = QPB * 128
                    qt = qtpool.tile([128, ND, w], f32r, tag="qtpass")
                    nc.sync.dma_start(qt[:], qr[:, :, pi * w:(pi + 1) * w])
                    qt_tiles[pi] = qt

            def issue_kt_slice(si):
                nc.sync.dma_start(
                    kt_all[:, :, si * KTW:(si + 1) * KTW],
                    kr[:, :, si * KTW:(si + 1) * KTW],
                )

            # ---- per-q-block stage emitters ------------------------------
            state = {}

            def emit_alloc(qb):
                st = state[qb] = {}
                asum_t = smallp.tile([128, 16], f32, tag="asum")
                st["asum"] = asum_t
                o_ps_t = psum_o.tile([128, D], f32, tag="ops")
                st["o_ps"] = o_ps_t
                st["p"] = {}

            def emit_qkt(qb, kt):
                st = state[qb]
                n_kc = qb + 1
                n_kt = (n_kc + 3) // 4
                qt = qt_tiles[qb // QPB]
                qoff = (qb % QPB) * 128
                # diagonal k-tile: shrink to the 256-multiple covering kspan
                if kt == n_kt - 1:
                    rem = n_kc - kt * 4  # 1..4 chunks of 128
                    w = 256 if rem <= 2 else 512 if rem == 4 else 384
                else:
                    w = KTW
                s_ps = psum_s.tile([128, KTW], f32, tag="s_ps")
                for dc in range(ND):
                    nc.tensor.matmul(
                        s_ps[:, :w],
                        qt[:, dc, qoff:qoff + 128],
                        kt_all[:, dc, kt * KTW:kt * KTW + w],
                        start=(dc == 0),
                        stop=(dc == ND - 1),
                    )
                if kt == n_kt - 1:
                    ri = qb - (n_kt - 1) * 4
                    nc.vector.tensor_add(
                        s_ps[:, :w], s_ps[:, :w], cmask[:, ri, :w]
                    )
                p_sb = pp.tile([128, KTW], f32r)
                nc.scalar.activation(
                    p_sb[:, :w], s_ps[:, :w], EXP,
                    bias=0.0, scale=SOFTMAX_SCALE,
                    accum_out=st["asum"][:, kt:kt + 1],
                )
                st["p"][kt] = p_sb

            def emit_ptpv(qb, kt):
                st = state[qb]
                n_kc = qb + 1
                p_sb = st["p"].pop(kt)
                o_ps = st["o_ps"]
                n_j = min(4, n_kc - kt * 4)
                pt_ps = psum_tr.tile([128, 512], f32r, tag="tr")
                pt_sb = ptp.tile([128, 512], f32r)
                for j in range(n_j):
                    nc.tensor.transpose(
                        pt_ps[:, j * 128:(j + 1) * 128],
                        p_sb[:, j * 128:(j + 1) * 128],
                        ident_r,
                    )
                nc.vector.tensor_copy(pt_sb[:, :n_j * 128], pt_ps[:, :n_j * 128])
                for j in range(n_j):
                    kc = kt * 4 + j
                    for h in range(2):
                        nc.tensor.matmul(
                            o_ps[:, h * 512:(h + 1) * 512],
                            pt_sb[:, j * 128:(j + 1) * 128],
                            v_all[:, kc, h * 512:(h + 1) * 512],
                            start=(kc == 0),
                            stop=(kc == n_kc - 1),
                        )

            def emit_fin(qb):
                st = state.pop(qb)
                n_kt = (qb + 4) // 4
                rsum = smallp.tile([128, 1], f32, tag="rsum")
                nc.vector.reduce_sum(rsum[:], st["asum"][:, :n_kt], axis=X)
                rinv = smallp.tile([128, 1], f32, tag="rinv")
                nc.vector.reciprocal(rinv[:], rsum[:])
                o_sb = osbp.tile([128, D], f32)
                for h in range(2):
                    nc.scalar.mul(
                        o_sb[:, h * 512:(h + 1) * 512],
                        st["o_ps"][:, h * 512:(h + 1) * 512],
                        rinv[:],
                    )
                    nc.sync.dma_start(
                        o_d[qb * 128:(qb + 1) * 128, h * 512:(h + 1) * 512],
                        o_sb[:, h * 512:(h + 1) * 512],
                    )

            def emit_qblock(qb):
                n_kt = (qb + 4) // 4
                emit_alloc(qb)
                for kt in range(n_kt):
                    emit_qkt(qb, kt)
                    if kt >= 1:
                        emit_ptpv(qb, kt - 1)
                emit_ptpv(qb, n_kt - 1)
                emit_fin(qb)

            # ---- merged pipeline -----------------------------------------
            issue_qt_pass(0)
            issue_kt_slice(0)
            nc.sync.dma_start(v_all[:, 0, :], v_d[0:128, :])
            issue_qt_pass(1)
            for kc in range(1, NKC):
                if kc <= 3:
                    issue_kt_slice(kc)
                nc.sync.dma_start(
                    v_all[:, kc, :], v_d[kc * 128:(kc + 1) * 128, :]
                )
                emit_qblock(kc - 1)
                if kc % QPB == 2:
                    # prefetch the next Q^T pass once the pass two back has
                    # retired (avoids a WAR stall on the SP DMA stream)
                    issue_qt_pass((kc + 2) // QPB)
            emit_qblock(NQ - 1)

    _split_waits(nc)
    return nc


def _np_reference(query, key, value, mask):
    """Host fallback for the general (non-all-ones) padding-mask case."""
    out = np.empty_like(query)
    tri = np.triu(np.ones((T, T), dtype=np.float32), 1) * 1e10
    for b in range(B):
        s = query[b] @ key[b].T
        s = s - tri
        s = s - (1.0 - mask[b])[None, :] * 1e10
        s = s * SOFTMAX_SCALE
        s = s - s.max(axis=-1, keepdims=True)
        p = np.exp(s)
        p = p / p.sum(axis=-1, keepdims=True)
        out[b] = p @ value[b]
    return out


def make_in_maps(query, key, value):
    """Per-core input dicts: batch b -> core b, with host-side Q^T/K^T."""
    maps = []
    for b in range(B):
        maps.append({
            "query_t": np.ascontiguousarray(query[b].T),
            "key_t": np.ascontiguousarray(key[b].T),
            "value": np.ascontiguousarray(value[b]),
        })
    return maps


def kernel(query, key, value, mask):
    query = np.asarray(query, dtype=np.float32)
    key = np.asarray(key, dtype=np.float32)
    value = np.asarray(value, dtype=np.float32)
    mask = np.asarray(mask, dtype=np.float32)

    if not np.all(mask == 1.0):
        return _np_reference(query, key, value, mask)

    from concourse.bass_utils import run_bass_kernel_spmd

    if "nc" not in _CACHE:
        _CACHE["nc"] = _build()
    nc = _CACHE["nc"]

    in_maps = make_in_maps(query, key, value)
    res = run_bass_kernel_spmd(nc, in_maps, core_ids=list(range(B)))
    out = np.stack([res.results[b]["out"] for b in range(B)], axis=0)
    return out.astype(np.float32)
